# revision 12
# baseline (speedup 1.0000x reference)
"""Trainium2 Bass kernel for nn_Attention (dense transformer attention block).

Computation (per reference):
  q = x @ wq.T; k = x @ wk.T; v = x @ wv.T       (GQA: 16 q heads, 4 kv heads)
  rope(q, k) with cos/sin from freqs (interleaved complex pairs)
  non-causal SDPA with softmax over keys, scale 1/sqrt(128)
  out = (probs @ v reshaped) @ wo.T

Sharding (8 cores): tensor-parallel over the 4 kv-head groups (TP=4; each
core gets 4 q heads + 1 kv head, wq/wk/wv column-sharded, wo row-sharded)
x data-parallel over batch (DP=2; 2 batches per core). Each core computes a
partial output [2, S, DIM]; the host sums the 4 TP partials per batch pair.

Device layout notes:
 - x is passed transposed per batch: xt[b] = x[b].T  [DIM, S] so that
   projections contract over DIM on the partition axis.
 - Q^T/K^T are computed in [head_dim, S] bf16 layout; per head the 128
   head-dim rows are permuted to [evens(64) | odds(64)] (done by permuting
   wq/wk rows on the host) so RoPE pairs are partition-contiguous halves.
 - cos/sin tables arrive partition-duplicated from the host as one
   [128, 2, S] tensor per batch (1 DMA descriptor).
 - DMA: descriptors are merged (whole x chunk / whole weight / whole output
   row-block per descriptor) because each descriptor costs ~600ns of issue
   on its queue and each queue walks packets at only ~140-200 GB/s.
   Inputs ride the sync queue; weights and half the outputs ride the
   scalar-engine queue so the two DGE walkers run in parallel.
 - phase B (attention) is a flat lag-2 software pipeline over global ticks
   t = (head, q-chunk, key-group): tick t emits the AV/colsum matmuls of
   tick t-2 and the finalize of the hqc that ended at t-3, so the PE never
   waits on the ACT exp latency (ACT is the bottleneck engine in phase B:
   (1024+352)/1.2ns per 2-key-tile exp vs ~985ns of PE work per tick).
 - PSUM: 2x score pairs (4 banks) + 2x AV banks + 2x colsum banks = 8.
   Phase A reuses the same tags (q into the pairs, K/V into the banks).
 - softmax denominators: 4 col-tiled ones-matmuls per 4 key tiles
   accumulate per-column partials; a selection-matrix matmul broadcasts
   their sum to 128 partitions (reusing the colsum bank); reciprocal on
   DVE; the AV output is scaled by it.
 - matmul operands are bf16 throughout (q/k/v/e/wo); accumulation fp32.
"""

import numpy as np
from contextlib import ExitStack

import ml_dtypes

import concourse.bacc as bacc
import concourse.tile as tile
from concourse import mybir
from concourse.bass_utils import run_bass_kernel_spmd
from concourse.masks import make_identity

F32 = mybir.dt.float32
F32R = mybir.dt.float32r
BF16 = mybir.dt.bfloat16

N_HEADS = 16
N_KV_HEADS = 4
DIM = 2048
HD = 128
B = 4
S_FULL = 2048
TP = 4            # tensor-parallel over kv-head groups
DP = 2            # data-parallel over batch
BPC = B // DP     # batches per core
HQ = N_HEADS // TP  # q heads per core
DK = DIM // 128   # contraction tiles over model dim
SCALE = float(1.0 / np.sqrt(HD))

_NC_CACHE = {}


def build_nc(s):
    sc_n = s // 512   # 512-wide s/q chunks
    kt_n = s // 128   # 128-wide key tiles
    st_n = s // 128   # 128-wide s tiles
    kg_n = kt_n // 2  # key groups (2 key tiles each) = ticks per hqc

    nc = bacc.Bacc("TRN2", target_bir_lowering=False, debug=False)
    xt = nc.dram_tensor("xt", [BPC, DIM, s], BF16, kind="ExternalInput")
    cst = nc.dram_tensor("cst", [BPC, 128, 2, s], F32, kind="ExternalInput")
    wqt = nc.dram_tensor("wqt", [DIM, HQ * HD], BF16, kind="ExternalInput")
    wkt = nc.dram_tensor("wkt", [DIM, HD], BF16, kind="ExternalInput")
    wvt = nc.dram_tensor("wvt", [DIM, HD], BF16, kind="ExternalInput")
    wot = nc.dram_tensor("wot", [HQ * HD, DIM], BF16, kind="ExternalInput")
    outp = nc.dram_tensor("outp", [BPC, s, DIM], BF16, kind="ExternalOutput")

    xt_v = xt.rearrange("b (t p) c -> b p t c", p=128)      # [BPC,128,DK,s]
    wq_v = wqt.rearrange("(dk p) c -> p dk c", p=128)       # [128,DK,512]
    wk_v = wkt.rearrange("(dk p) c -> p dk c", p=128)       # [128,DK,128]
    wv_v = wvt.rearrange("(dk p) c -> p dk c", p=128)       # [128,DK,128]
    wo_v = wot.rearrange("(h p) c -> p h c", p=128)         # [128,HQ,DIM]

    with ExitStack() as ctx:
        ctx.enter_context(
            nc.allow_low_precision(reason="bf16 matmul pipeline by design")
        )
        tc = ctx.enter_context(tile.TileContext(nc))

        singles = ctx.enter_context(tc.tile_pool(name="singles", bufs=1))
        qt_pool = ctx.enter_context(tc.tile_pool(name="qt", bufs=1))
        kt_pool = ctx.enter_context(tc.tile_pool(name="ktp", bufs=1))
        v_pool = ctx.enter_context(tc.tile_pool(name="vp", bufs=1))
        e_pool = ctx.enter_context(tc.tile_pool(name="ep", bufs=2))
        ot_pool = ctx.enter_context(tc.tile_pool(name="otp", bufs=1))
        cs_pool = ctx.enter_context(tc.tile_pool(name="csp", bufs=2))
        xt_pool = ctx.enter_context(tc.tile_pool(name="xtp", bufs=2))
        tmp_pool = ctx.enter_context(tc.tile_pool(name="tmp", bufs=2))
        vt_pool = ctx.enter_context(tc.tile_pool(name="vtp", bufs=2))
        csum_pool = ctx.enter_context(tc.tile_pool(name="csum", bufs=2))
        rcp_pool = ctx.enter_context(tc.tile_pool(name="rcp", bufs=2))
        av_sb_pool = ctx.enter_context(tc.tile_pool(name="avsb", bufs=2))
        orow_pool = ctx.enter_context(tc.tile_pool(name="orow", bufs=2))

        psum = ctx.enter_context(tc.tile_pool(name="psum", bufs=1, space="PSUM"))

        def ps_pair(name):
            return psum.tile([128, 2, 512], F32, tag="pair", bufs=2, name=name)

        def ps_bank(name, tag):
            return psum.tile([128, 512], F32, tag=tag, bufs=2, name=name)

        # ---- constants (also warmup operands) ----
        ones32_bf = singles.tile([128, 32], BF16)
        nc.vector.memset(ones32_bf, 1.0)
        sel4_f = singles.tile([128, 128], F32)
        nc.vector.memset(sel4_f, 0.0)
        for j in range(4):
            nc.vector.memset(sel4_f[32 * j:32 * j + 1, :], 1.0)
        sel4 = singles.tile([128, 128], F32R)
        nc.vector.tensor_copy(sel4, sel4_f)
        ident = singles.tile([128, 128], F32)
        make_identity(nc, ident)

        # ---- weights: one descriptor each on the scalar queue; wq split
        # in two so the first q matmul isn't gated on the full 2MB ----
        wk_sb = singles.tile([128, DK, HD], BF16)
        wv_sb = singles.tile([128, DK, HD], BF16)
        wq_sb = singles.tile([128, DK, HQ * HD], BF16)
        nc.scalar.dma_start(out=wk_sb, in_=wk_v)
        nc.scalar.dma_start(out=wv_sb, in_=wv_v)
        nc.scalar.dma_start(out=wq_sb[:, : DK // 2], in_=wq_v[:, : DK // 2])
        nc.scalar.dma_start(out=wq_sb[:, DK // 2:], in_=wq_v[:, DK // 2:])

        wo_sb = singles.tile([128, HQ, DIM], BF16)
        wo_loaded = [False]

        def load_wo():
            if not wo_loaded[0]:
                nc.sync.dma_start(out=wo_sb, in_=wo_v)
                wo_loaded[0] = True

        # ---- x chunks + cos/sin: one descriptor per chunk on sync ----
        xq = {}

        def issue_x(b, sc):
            t = xt_pool.tile([128, DK, 512], BF16, tag="xc", name=f"xc{b}{sc}")
            nc.sync.dma_start(out=t, in_=xt_v[b, :, :, sc * 512:(sc + 1) * 512])
            xq[(b, sc)] = t

        csq = {}

        def issue_cs(b):
            t = cs_pool.tile([128, 2, s], F32, tag="cs", name=f"cs{b}")
            nc.sync.dma_start(out=t, in_=cst[b])
            csq[b] = t

        # ---- warmup: dependency-free matmuls keep PE busy (and the HAM
        # clock un-throttled) while the first weight/x DMAs land ----
        warm_ps = ps_pair("warmps")
        issue_x(0, 0)
        issue_cs(0)
        for wi in range(24):
            nc.tensor.matmul(
                warm_ps[:, wi % 2, 0:128], ident, sel4_f, start=True, stop=True
            )
        if sc_n > 1:
            issue_x(0, 1)

        copy_flip = [0]

        def copy_any(dst, src):
            # alternate psum->sbuf copies between ScalarE and VectorE
            if copy_flip[0] % 2 == 0:
                nc.scalar.copy(dst, src)
            else:
                nc.vector.tensor_copy(dst, src)
            copy_flip[0] += 1

        for b in range(BPC):
            # ================= phase A: projections + rope =================
            cs2 = csq[b]
            qt = qt_pool.tile([128, HQ, s], BF16)
            kt = kt_pool.tile([128, s], BF16)
            vsb = v_pool.tile([128, st_n, HD], BF16)

            def rope(src_ps, dst_r, dst_i, ss):
                # r' = qr*cos - qi*sin ; i' = qr*sin + qi*cos
                # p1 = src*cos (SBUF); then src *= sin in place (PSUM).
                p1 = tmp_pool.tile([128, 512], F32, tag="tmp", name="p1")
                nc.vector.tensor_mul(p1, src_ps, cs2[:, 0, ss])
                nc.vector.tensor_mul(src_ps, src_ps, cs2[:, 1, ss])
                nc.vector.tensor_sub(dst_r, p1[0:64, :], src_ps[64:128, :])
                nc.vector.tensor_add(dst_i, src_ps[0:64, :], p1[64:128, :])

            for sc in range(sc_n):
                ss = slice(sc * 512, (sc + 1) * 512)
                if sc + 2 < sc_n:
                    issue_x(b, sc + 2)
                xtile = xq[(b, sc)]
                kps = ps_bank(f"kps{b}{sc}", "av")
                vps = ps_bank(f"vps{b}{sc}", "cs")
                for dk in range(DK):
                    nc.tensor.matmul(
                        kps, wk_sb[:, dk, :], xtile[:, dk, :],
                        start=(dk == 0), stop=(dk == DK - 1),
                    )
                rope(kps, kt[0:64, ss], kt[64:128, ss], ss)
                for dk in range(DK):
                    nc.tensor.matmul(
                        vps, wv_sb[:, dk, :], xtile[:, dk, :],
                        start=(dk == 0), stop=(dk == DK - 1),
                    )
                vt_sb = vt_pool.tile([128, 512], F32, tag="vt", name="vt")
                nc.scalar.copy(vt_sb, vps)
                qps = [None, None]
                for m in range(HQ):
                    if m % 2 == 0:
                        qps[m // 2] = ps_pair(f"qps{b}{sc}{m}")
                    for dk in range(DK):
                        nc.tensor.matmul(
                            qps[m // 2][:, m % 2, :],
                            wq_sb[:, dk, m * HD:(m + 1) * HD],
                            xtile[:, dk, :],
                            start=(dk == 0), stop=(dk == DK - 1),
                        )
                    if m == 0:
                        # V transpose slots in while ACT's vt copy has
                        # finished and PE streams head-1 q matmuls next
                        for i in range(4):
                            vtr = vps[:, i * 128:(i + 1) * 128]
                            nc.tensor.transpose(
                                vtr, vt_sb[:, i * 128:(i + 1) * 128], ident
                            )
                            nc.scalar.copy(vsb[:, sc * 4 + i, :], vtr)
                    rope(
                        qps[m // 2][:, m % 2, :],
                        qt[0:64, m, ss], qt[64:128, m, ss], ss,
                    )

            # ============ phase B: flat lag-2 attention pipeline ============
            if b == 0:
                load_wo()
            if b + 1 < BPC:
                issue_x(b + 1, 0)
                if sc_n > 1:
                    issue_x(b + 1, 1)
                issue_cs(b + 1)
            outT = ot_pool.tile([128, HQ, s], BF16)

            hqcs = [(h, qc) for h in range(HQ) for qc in range(sc_n)]
            n_ticks = len(hqcs) * kg_n
            e_tiles = {}
            av_tiles = {}
            cs_tiles = {}
            fin_sb = {}

            def qslice(i):
                return slice(hqcs[i][1] * 512, hqcs[i][1] * 512 + 512)

            for T in range(n_ticks + 3):
                # --- lagged AV / colsum / finalize_a for tick T-2 ---
                L = T - 2
                if 0 <= L < n_ticks:
                    li, lkg = divmod(L, kg_n)
                    le_t = e_tiles[li]
                    if lkg == 0:
                        av_tiles[li] = ps_bank(f"av{b}_{li}", "av")
                    av = av_tiles[li]
                    for j in range(2):
                        ktile = 2 * lkg + j
                        nc.tensor.matmul(
                            av, vsb[:, ktile, :], le_t[:, ktile, :],
                            start=(ktile == 0), stop=(ktile == kt_n - 1),
                        )
                    if lkg % 2 == 1:
                        i4 = lkg // 2
                        if i4 == 0:
                            cs_tiles[li] = ps_bank(f"cs{b}_{li}", "cs")
                        cst_ = cs_tiles[li]
                        for cj in range(4):
                            ktile = 4 * i4 + cj
                            nc.tensor.matmul(
                                cst_[32 * cj:32 * (cj + 1), :],
                                ones32_bf, le_t[:, ktile, :],
                                start=(i4 == 0), stop=(i4 == kg_n // 2 - 1),
                                tile_position=(0, 32 * cj),
                            )
                    if lkg == kg_n - 1:
                        # finalize_a: drain AV + colsum partials to SBUF
                        csum = csum_pool.tile(
                            [128, 512], F32R, tag="csum", name="csum"
                        )
                        nc.vector.tensor_copy(csum, cs_tiles[li])
                        av_sb = av_sb_pool.tile(
                            [128, 512], BF16, tag="avsb", name="avsb"
                        )
                        nc.vector.tensor_copy(av_sb, av)
                        fin_sb[li] = (csum, av_sb)
                # --- finalize_b for the hqc that ended at tick T-3 ---
                F = T - 3
                if F >= 0 and F % kg_n == kg_n - 1:
                    fi = F // kg_n
                    csum, av_sb = fin_sb.pop(fi)
                    cst_ = cs_tiles.pop(fi)
                    nc.tensor.matmul(cst_, sel4, csum, start=True, stop=True)
                    rcp = rcp_pool.tile([128, 512], F32, tag="rcp", name="rcp")
                    nc.vector.reciprocal_approx_fast(out=rcp, in_=cst_)
                    nc.vector.tensor_mul(
                        outT[:, hqcs[fi][0], qslice(fi)], av_sb, rcp
                    )
                    del av_tiles[fi]
                    del e_tiles[fi]
                # --- scores + exp for tick T ---
                if T < n_ticks:
                    ti, kg = divmod(T, kg_n)
                    h = hqcs[ti][0]
                    qs = qslice(ti)
                    if kg == 0:
                        e_tiles[ti] = e_pool.tile(
                            [128, kt_n, 512], BF16, tag="et", name=f"et{b}_{ti}"
                        )
                    scp = ps_pair(f"sc{b}_{T}")
                    for j in range(2):
                        ktile = 2 * kg + j
                        nc.tensor.matmul(
                            scp[:, j, :],
                            kt[:, ktile * 128:(ktile + 1) * 128],
                            qt[:, h, qs],
                            start=True, stop=True,
                        )
                    nc.scalar.activation(
                        out=e_tiles[ti][:, 2 * kg:2 * kg + 2, :],
                        in_=scp,
                        func=mybir.ActivationFunctionType.Exp,
                        scale=SCALE,
                    )

            # ================= phase C: output projection =================
            for scb in range(st_n):
                sb_ = slice(scb * 128, (scb + 1) * 128)
                orow = orow_pool.tile([128, DIM], BF16, tag="orow", name="orow")
                for dcp in range(2):
                    ops_ = ps_pair(f"ops{b}{scb}{dcp}")
                    for jj in range(2):
                        dc = 2 * dcp + jj
                        for h2 in range(HQ):
                            nc.tensor.matmul(
                                ops_[:, jj, :],
                                outT[:, h2, sb_],
                                wo_sb[:, h2, dc * 512:(dc + 1) * 512],
                                start=(h2 == 0), stop=(h2 == HQ - 1),
                            )
                    copy_any(
                        orow[:, dcp * 1024:(dcp + 1) * 1024],
                        ops_.rearrange("p a c -> p (a c)"),
                    )
                # alternate output descriptors across the two DGE queues
                if scb % 2 == 0:
                    nc.sync.dma_start(out=outp[b, sb_, :], in_=orow)
                else:
                    nc.scalar.dma_start(out=outp[b, sb_, :], in_=orow)

    nc.compile()
    return nc


_PERM = np.concatenate([np.arange(0, HD, 2), np.arange(1, HD, 2)])


def _prep_inputs(x, freqs, wq, wk, wv, wo, s):
    """Build the 8 per-core input maps."""
    in_maps = []
    xt_dp = []
    cs_dp = []
    for dp in range(DP):
        bs = slice(dp * BPC, (dp + 1) * BPC)
        xt_dp.append(
            np.ascontiguousarray(x[bs].transpose(0, 2, 1)).astype(ml_dtypes.bfloat16)
        )
        cos_t = np.cos(freqs[bs]).transpose(0, 2, 1)   # [BPC, 64, s]
        sin_t = np.sin(freqs[bs]).transpose(0, 2, 1)
        cs = np.stack([cos_t, sin_t], axis=2)          # [BPC, 64, 2, s]
        cs = np.concatenate([cs, cs], axis=1)          # [BPC, 128, 2, s]
        cs_dp.append(np.ascontiguousarray(cs.astype(np.float32)))
    for core in range(8):
        g = core % TP
        dp = core // TP
        wq_g = wq[g * HQ * HD:(g + 1) * HQ * HD]  # [512, DIM]
        wq_p = wq_g.reshape(HQ, HD, DIM)[:, _PERM, :].reshape(HQ * HD, DIM)
        wk_g = wk[g * HD:(g + 1) * HD][_PERM]      # [128, DIM]
        wv_g = wv[g * HD:(g + 1) * HD]             # [128, DIM]
        wo_g = wo[:, g * HQ * HD:(g + 1) * HQ * HD]  # [DIM, 512]
        in_maps.append(
            {
                "xt": xt_dp[dp],
                "cst": cs_dp[dp],
                "wqt": np.ascontiguousarray(wq_p.T).astype(ml_dtypes.bfloat16),
                "wkt": np.ascontiguousarray(wk_g.T).astype(ml_dtypes.bfloat16),
                "wvt": np.ascontiguousarray(wv_g.T).astype(ml_dtypes.bfloat16),
                "wot": np.ascontiguousarray(wo_g.T).astype(ml_dtypes.bfloat16),
            }
        )
    return in_maps


_LAST = {}


def _run(x, freqs, wq, wk, wv, wo, s):
    x = np.asarray(x, dtype=np.float32)
    freqs = np.asarray(freqs, dtype=np.float32)
    wq = np.asarray(wq, dtype=np.float32)
    wk = np.asarray(wk, dtype=np.float32)
    wv = np.asarray(wv, dtype=np.float32)
    wo = np.asarray(wo, dtype=np.float32)

    if s not in _NC_CACHE:
        _NC_CACHE[s] = build_nc(s)
    nc = _NC_CACHE[s]
    in_maps = _prep_inputs(x, freqs, wq, wk, wv, wo, s)
    res = run_bass_kernel_spmd(nc, in_maps, core_ids=list(range(8)))
    _LAST["nc"] = nc
    _LAST["in_maps"] = in_maps

    out = np.empty((B, s, DIM), dtype=np.float32)
    for dp in range(DP):
        acc = res.results[dp * TP]["outp"].astype(np.float32)
        for g in range(1, TP):
            acc += res.results[dp * TP + g]["outp"].astype(np.float32)
        out[dp * BPC:(dp + 1) * BPC] = acc
    return out


def kernel(x, freqs, wq, wk, wv, wo):
    return _run(x, freqs, wq, wk, wv, wo, S_FULL)
